# revision 1
# baseline (speedup 1.0000x reference)
"""Trainium2 Bass kernel for nn_Affinity (gnn_message_passing).

M[(a,b),(c,d)] = sum_{j,i} H2[a,j]H2[c,j] H1[b,i]H1[d,i] Me[j,i] + diag(Mp).

M is [5184, 5184] f32, block-sparse: block (a,c) is nonzero only when a==c or
(a,c) is an edge of graph 2. Strategy: shard output rows (a-bands) across the
8 cores; on each core compute only the nonzero [72,72] blocks via matmuls
(fully dense math is ~6x more PE work), zero-fill the core's output slab with
big HWDGE DMAs, and scatter the computed blocks with one indirect (SWDGE) DMA
per bundle using host-precomputed flat indices. All index-derived tables
(incidence matrices, selection matrices, scatter indices) are host-built and
passed as per-core inputs; every floating-point op runs on device.
"""
import sys
sys.path.insert(0, '/opt/trn_rl_repo')
import math
import numpy as np

N = 72
E = 288
D = 64
NC = 8
W = N * N          # 5184
GROUP_ROWS = 3 * N  # 216 rows per output tensor
PAD_IDX = 2 ** 30

F32 = None
I32 = None


def _split_waits(nc, limit=1):
    """This walrus build rejects instructions with >limit sem waits; move the
    excess onto same-engine NoOps inserted immediately before (same bb order =
    same engine program order, so semantics are preserved)."""
    import concourse.mybir as mybir
    for f in nc.m.functions:
        for bb in f.blocks:
            new_insts = []
            for inst in bb.instructions:
                si = inst.sync_info
                waits = list(si.on_wait) if si and si.on_wait else []
                if len(waits) > limit:
                    extra, keep = waits[:-limit], waits[-limit:]
                    for i in range(0, len(extra), limit):
                        nop = mybir.InstNoOp(
                            name=nc.get_next_instruction_name(),
                            engine=inst.engine, ins=[], outs=[],
                            sync_info=mybir.SyncInfo(
                                on_wait=extra[i:i + limit], on_update=[]),
                        )
                        nc.register_instruction(nop)
                        new_insts.append(nop)
                    si.on_wait = keep
                new_insts.append(inst)
            bb.instructions[:] = new_insts


def _incidence(src, dst):
    H = np.zeros((N, E), np.float32)
    H[src, np.arange(E)] = 1.0
    H[dst, np.arange(E)] = 1.0
    return H


def _plan_assignment(src2, dst2):
    """Balance a-bands across cores and 3 groups/core; K = bundle capacity."""
    nbrs = [set() for _ in range(N)]
    for s, d in zip(src2, dst2):
        nbrs[int(s)].add(int(d))
        nbrs[int(d)].add(int(s))
    deg = [len(x) for x in nbrs]
    order = sorted(range(N), key=lambda a: -deg[a])
    cores = [[] for _ in range(NC)]
    loads = [0] * NC
    for a in order:
        c = min((c for c in range(NC) if len(cores[c]) < 9), key=lambda c: loads[c])
        cores[c].append(a)
        loads[c] += deg[a]
    plans = []
    K = 2
    for c in range(NC):
        bands_sorted = sorted(cores[c], key=lambda a: -deg[a])
        groups = [[] for _ in range(3)]
        gl = [0] * 3
        for a in bands_sorted:
            g = min((g for g in range(3) if len(groups[g]) < 3), key=lambda g: gl[g])
            groups[g].append(a)
            gl[g] += deg[a]
        for g in range(3):
            K = max(K, 1 + math.ceil(gl[g] / 3))
        plans.append(groups)
    return plans, nbrs, K


def _build_tables(plans, nbrs, K, H2):
    """Per-core (SELT [E,9K] f32, IDX [9K,1] i32, OH [N,9] f32, band_list).

    IDX[slot] = flat element offset of the block's row in the BLOCK-TILED
    group tensor [216, 5184]: row (la_in_group*72 + c), i.e. offset
    (la_in_group*72 + c) * 5184. Padding slots get PAD_IDX (skipped via
    bounds_check)."""
    tables = []
    for c in range(NC):
        groups = plans[c]
        band_list = groups[0] + groups[1] + groups[2]
        SELT = np.zeros((E, 9 * K), np.float32)
        IDX = np.full((9 * K, 1), PAD_IDX, np.int32)
        OH = np.zeros((N, 9), np.float32)
        for l in range(9):
            a = band_list[l]
            OH[a, l] = 1.0
            lb = l % 3
            SELT[:, l * K] = H2[a] * H2[a]
            IDX[l * K, 0] = (lb * N + a) * W
        for g in range(3):
            blocks = []
            for j, a in enumerate(groups[g]):
                for cc in sorted(nbrs[a]):
                    blocks.append((a, cc, j))
            slots = [(3 * g + i, k) for k in range(1, K) for i in range(3)]
            assert len(blocks) <= len(slots)
            for (a, cc, j), (l, k) in zip(blocks, slots):
                SELT[:, l * K + k] = H2[a] * H2[cc]
                IDX[l * K + k, 0] = (j * N + cc) * W
        tables.append((SELT, IDX, OH, band_list))
    return tables


def _pack_fields(K):
    return [("U1T", D, 72), ("U2T", D, 72), ("OH", 72, 9)]


def _packb_fields(K):
    KC = 9 * K

    def pad(w):
        return (w + 63) // 64 * 64
    return [("H1T0", 96, pad(N)), ("H1T1", 96, pad(N)), ("H1T2", 96, pad(N)),
            ("F1", 72, pad(D)), ("H1", 72, pad(E)), ("F2", 72, pad(D)),
            ("L1T", D, pad(D)), ("L2T", D, pad(D)), ("S2", 72, pad(E)),
            ("D2M", 72, pad(E)),
            ("SELT0", 96, pad(KC)), ("SELT1", 96, pad(KC)), ("SELT2", 96, pad(KC))]


def _pack_offsets(K):
    offs = {}
    pw = 0
    for nm, r, w in _pack_fields(K):
        offs[nm] = pw
        pw += w
    return offs


def _pack_width(K):
    return sum(w for _, _, w in _pack_fields(K))


def _packb_offsets(K):
    offs = {}
    pw = 0
    for nm, r, w in _packb_fields(K):
        offs[nm] = pw
        pw += w
    return offs


def _packb_width(K):
    return sum(w for _, _, w in _packb_fields(K))


def _build_nc(K, zero_fill=False):
    import concourse.bass as bass
    import concourse.mybir as mybir
    import concourse.tile as tile
    from concourse.masks import make_identity

    F32 = mybir.dt.float32
    BF16 = mybir.dt.bfloat16
    I32 = mybir.dt.int32
    KC = 9 * K

    nc = bass.Bass()
    pack_d = nc.declare_dram_parameter("PACK", [96, _pack_width(K)], F32,
                                       isOutput=False)
    packb_d = nc.declare_dram_parameter("PACKB", [96, _packb_width(K)], BF16, isOutput=False)
    idx_d = nc.declare_dram_parameter("IDX3", [3 * K, 3], I32, isOutput=False)
    out_d = [nc.declare_dram_parameter(f"out{g}", [GROUP_ROWS, W], F32, isOutput=True)
             for g in range(3)]
    # internal DRAM bounce for the b<->k axis swap, stored k-major (the
    # permuted, small-segment write happens early and hidden; the read back
    # into block-per-partition SBUF layout is fully contiguous)
    scratch_d = [nc.dram_tensor(f"restage_scratch{g}", [3 * K, N * N], F32)
                 for g in range(3)]

    offs = _pack_offsets(K)
    offsb = _packb_offsets(K)

    with tile.TileContext(nc) as tc:
        with tc.tile_pool(name="cst", bufs=1) as cst, \
             tc.tile_pool(name="wrk", bufs=2) as wrk, \
             tc.tile_pool(name="stg", bufs=3) as stg, \
             tc.tile_pool(name="ps", bufs=2, space="PSUM") as ps, \
             tc.tile_pool(name="psb", bufs=5, space="PSUM") as psb:

            if zero_fill:
                zt = cst.tile([128, N * W // 128], F32)
                nc.vector.memset(zt[:], 0.0)
                for g in range(3):
                    for j in range(3):
                        dst = out_d[g][j * N:(j + 1) * N, :] \
                            .rearrange("b w -> (b w)") \
                            .rearrange("(p f) -> p f", p=128)
                        eng = nc.sync if (g * 3 + j) % 2 == 0 else nc.scalar
                        eng.dma_start(out=dst, in_=zt[:])

            # ---- input loads: 3 DMAs total ----
            pk = cst.tile([96, _pack_width(K)], F32)
            nc.sync.dma_start(out=pk[:], in_=pack_d[:])
            pkb = cst.tile([96, _packb_width(K)], BF16)
            nc.scalar.dma_start(out=pkb[:], in_=packb_d[:])
            idx3 = cst.tile([3 * K, 3], I32)
            nc.sync.dma_start(out=idx3[:], in_=idx_d[:])

            def fld(nm, r, w):
                return pk[0:r, offs[nm]:offs[nm] + w]

            def fldb(nm, r, w):
                return pkb[0:r, offsb[nm]:offsb[nm] + w]
            u1t = fld("U1T", D, 72)
            u2t = fld("U2T", D, 72)
            oh = fld("OH", 72, 9)
            f1 = fldb("F1", 72, D)
            h1 = fldb("H1", 72, E)
            f2 = fldb("F2", 72, D)
            l1t = fldb("L1T", D, D)
            l2t = fldb("L2T", D, D)
            s2 = fldb("S2", 72, E)
            d2m = fldb("D2M", 72, E)
            selt = [fldb(f"SELT{jc}", 96, KC) for jc in range(3)]
            h1tb = [fldb(f"H1T{ic}", 96, N) for ic in range(3)]

            ident = cst.tile([N, N], F32)
            make_identity(nc, ident[:])
            identb = cst.tile([N, N], BF16)
            nc.gpsimd.tensor_copy(out=identb[:], in_=ident[:])

            # ---- Mp / diag tiles ----
            mp_p = ps.tile([N, N], F32, tag="mmp")
            nc.tensor.matmul(out=mp_p[:], lhsT=u1t, rhs=u2t, start=True, stop=True)
            mp = wrk.tile([N, N], F32, tag="mp")
            nc.vector.tensor_copy(out=mp[:], in_=mp_p[:])
            mpsel_p = ps.tile([N, 9], F32, tag="mmp")
            nc.tensor.matmul(out=mpsel_p[:], lhsT=mp[:], rhs=oh, start=True, stop=True)
            mpsel = wrk.tile([N, 9], F32, tag="mpsel")
            nc.vector.tensor_copy(out=mpsel[:], in_=mpsel_p[:])
            dls = []
            for l in range(9):
                dlb = wrk.tile([N, N], BF16, name=f"dlb{l}", tag=f"dlb{l}")
                nc.scalar.activation(
                    out=dlb[:], in_=ident[:],
                    func=mybir.ActivationFunctionType.Copy,
                    scale=mpsel[:, l:l + 1])
                dls.append(dlb)

            # ---- Me / w chain ----
            r1t = wrk.tile([D, D], BF16, tag="r1t")
            nc.vector.tensor_relu(out=r1t[:], in_=l1t)
            r2t = wrk.tile([D, D], BF16, tag="r2t")
            nc.vector.tensor_relu(out=r2t[:], in_=l2t)

            fs_p = ps.tile([D, E], F32, tag="mmp")
            nc.tensor.matmul(out=fs_p[:], lhsT=f2, rhs=s2, start=True, stop=True)
            fs = wrk.tile([D, E], BF16, tag="fs")
            nc.vector.tensor_copy(out=fs[:], in_=fs_p[:])
            fd_p = ps.tile([D, E], F32, tag="mmp")
            nc.tensor.matmul(out=fd_p[:], lhsT=f2, rhs=d2m, start=True, stop=True)
            fdt = wrk.tile([D, E], BF16, tag="fdt")
            nc.vector.tensor_copy(out=fdt[:], in_=fd_p[:])

            z1t_p = ps.tile([D, E], F32, tag="mmp")
            nc.tensor.matmul(out=z1t_p[:], lhsT=f1, rhs=h1, start=True, stop=True)
            z1t = wrk.tile([D, E], BF16, tag="z1t")
            nc.vector.tensor_copy(out=z1t[:], in_=z1t_p[:])

            vv_p = ps.tile([D, E], F32, tag="mmp")
            nc.tensor.matmul(out=vv_p[:], lhsT=r1t[:], rhs=fs[:], start=True, stop=False)
            nc.tensor.matmul(out=vv_p[:], lhsT=r2t[:], rhs=fdt[:], start=False, stop=True)
            vv = wrk.tile([D, E], BF16, tag="vv")
            nc.vector.tensor_copy(out=vv[:], in_=vv_p[:])

            me = []
            for jc in range(3):
                me_p = ps.tile([96, E], F32, tag="mmp")
                nc.tensor.matmul(out=me_p[:], lhsT=z1t[:, 96 * jc:96 * (jc + 1)],
                                 rhs=vv[:], start=True, stop=True)
                me_c = wrk.tile([96, E], BF16, name=f"me{jc}", tag=f"me{jc}")
                nc.vector.tensor_copy(out=me_c[:], in_=me_p[:])
                me.append(me_c)

            wt = []
            for ic in range(3):
                wt_p = ps.tile([96, KC], F32, tag="mmp")
                for jc in range(3):
                    nc.tensor.matmul(out=wt_p[:], lhsT=me[jc][:, 96 * ic:96 * (ic + 1)],
                                     rhs=selt[jc], start=(jc == 0), stop=(jc == 2))
                wt_c = wrk.tile([96, KC], BF16, name=f"wtb{ic}", tag=f"wtb{ic}")
                nc.scalar.copy(out=wt_c[:], in_=wt_p[:])
                wt.append(wt_c)

            # ---- Stage A: 9 merged rhs builds (bundle-triple x ic),
            # split DVE (6) / GpSimd (3) ----
            rc3 = []
            for t in range(3):
                row = []
                for ic in range(3):
                    rc = cst.tile([96, 3 * K * N], BF16, name=f"rc{t}_{ic}",
                                  tag=f"rc{t}_{ic}")
                    eng = nc.vector  # gpsimd SBUF access locks out DVE
                    eng.tensor_tensor(
                        out=rc[:].rearrange("p (k d) -> p k d", d=N),
                        in0=wt[ic][:, 3 * K * t:3 * K * (t + 1)][:, :, None]
                            .to_broadcast([96, 3 * K, N]),
                        in1=h1tb[ic][:, None, :].to_broadcast([96, 3 * K, N]),
                        op=mybir.AluOpType.mult)
                    row.append(rc)
                rc3.append(row)

            # ---- Stage B: block matmuls + staged copies (ACT) ----
            stageds = []
            ngroups = [(s, min(s + 7, K)) for s in range(0, K, 7)]
            for l in range(9):
                staged = stg.tile([N, K * N], F32, name=f"staged{l}", tag=f"staged{l}")
                for (ks, ke) in ngroups:
                    bp = psb.tile([N, (ke - ks) * N], F32, tag="bp")
                    for ic in range(3):
                        last = (ic == 2) and ks != 0
                        nc.tensor.matmul(
                            out=bp[:], lhsT=h1tb[ic],
                            rhs=rc3[l // 3][ic][:, ((l % 3) * K + ks) * N:
                                                ((l % 3) * K + ke) * N],
                            start=(ic == 0), stop=last)
                    if ks == 0:
                        nc.tensor.matmul(out=bp[:, 0:N], lhsT=identb[:],
                                         rhs=dls[l][:], start=False, stop=True)
                    nc.scalar.copy(out=staged[:, ks * N:ke * N], in_=bp[:])
                stageds.append(staged)

            # ---- Stage C/D: per group, write the three bundles' staged
            # tiles to the group's scratch (k-major: the permuted small-segment
            # writes spread across all three DMA paths), then a contiguous
            # read-back and the group's indirect scatter, pipelined per group.
            restaged = cst.tile([9 * K, N * N], F32)
            rings = (nc.sync, nc.scalar, nc.gpsimd)
            for g in range(3):
                for j in range(3):
                    l = 3 * g + j
                    rings[j].dma_start(
                        out=scratch_d[g][j * K:(j + 1) * K, :].rearrange(
                            "k (b d) -> b k d", d=N),
                        in_=stageds[l][:].rearrange("b (k d) -> b k d", d=N))
            for g in range(3):
                rings[g % 2].dma_start(
                    out=restaged[3 * K * g:3 * K * (g + 1), :],
                    in_=scratch_d[g][:])
                nc.gpsimd.indirect_dma_start(
                    out=out_d[g][:],
                    out_offset=bass.IndirectOffsetOnAxis(ap=idx3[:, g:g + 1], axis=1),
                    in_=restaged[3 * K * g:3 * K * (g + 1), :],
                    in_offset=None,
                    bounds_check=GROUP_ROWS * W - 1,
                    oob_is_err=False)

    _split_waits(nc)
    return nc


def _prepare(inputs):
    import ml_dtypes
    ins = {k: np.asarray(v) for k, v in inputs.items()}
    F1 = ins["F1"].astype(np.float32)
    F2 = ins["F2"].astype(np.float32)
    U1 = ins["U1"].astype(np.float32)
    U2 = ins["U2"].astype(np.float32)
    l1 = ins["lamda1"].astype(np.float32)
    l2 = ins["lamda2"].astype(np.float32)
    src1 = ins["src1"].astype(np.int64)
    dst1 = ins["dst1"].astype(np.int64)
    src2 = ins["src2"].astype(np.int64)
    dst2 = ins["dst2"].astype(np.int64)

    H1 = _incidence(src1, dst1)
    H2 = _incidence(src2, dst2)
    S2 = np.zeros((N, E), np.float32)
    S2[src2, np.arange(E)] = 1.0
    D2M = np.zeros((N, E), np.float32)
    D2M[dst2, np.arange(E)] = 1.0

    plans, nbrs, K = _plan_assignment(src2, dst2)
    tables = _build_tables(plans, nbrs, K, H2)

    offs = _pack_offsets(K)
    offsb = _packb_offsets(K)
    base = np.zeros((96, _pack_width(K)), np.float32)

    def put(nm, arr):
        r, w = arr.shape
        base[0:r, offs[nm]:offs[nm] + w] = arr
    put("U1T", np.ascontiguousarray(U1.T))
    put("U2T", np.ascontiguousarray(U2.T))
    PACKB = np.zeros((96, _packb_width(K)), ml_dtypes.bfloat16)

    def putb(nm, arr):
        r, w = arr.shape
        PACKB[0:r, offsb[nm]:offsb[nm] + w] = arr.astype(ml_dtypes.bfloat16)
    for ic in range(3):
        putb(f"H1T{ic}", H1.T[96 * ic:96 * (ic + 1), :])
    putb("F1", F1)
    putb("H1", H1)
    putb("F2", F2)
    putb("L1T", np.ascontiguousarray(l1.T))
    putb("L2T", np.ascontiguousarray(l2.T))
    putb("S2", S2)
    putb("D2M", D2M)

    in_maps = []
    band_lists = []
    for c in range(NC):
        SELT, IDX, OH, band_list = tables[c]
        pack = base.copy()
        pack[0:72, offs["OH"]:offs["OH"] + 9] = OH
        packb = PACKB.copy()
        for jc in range(3):
            arr = SELT[96 * jc:96 * (jc + 1), :]
            packb[0:96, offsb[f"SELT{jc}"]:offsb[f"SELT{jc}"] + arr.shape[1]] = \
                arr.astype(ml_dtypes.bfloat16)
        IDX3 = np.ascontiguousarray(IDX.reshape(3, 3 * K).T).astype(np.int32)
        in_maps.append({"PACK": pack, "PACKB": packb, "IDX3": IDX3})
        band_lists.append(band_list)
    return in_maps, band_lists, K


_CACHE = {}


def kernel(**inputs):
    from concourse.bass_utils import run_bass_kernel_spmd

    in_maps, band_lists, K = _prepare(inputs)
    nc = _CACHE.get(K)
    if nc is None:
        nc = _build_nc(K)
        _CACHE[K] = nc
    res = run_bass_kernel_spmd(nc, in_maps, list(range(NC)))
    M = np.empty((N * N, N * N), np.float32)
    for c in range(NC):
        outs = res.results[c]
        for l in range(9):
            a = band_lists[c][l]
            g, j = l // 3, l % 3
            # out_g is block-tiled: [3(j), 72(c), 72(b), 72(d)]
            band = outs[f"out{g}"].reshape(3, N, N, N)[j]          # [c, b, d]
            M[a * N:(a + 1) * N, :] = band.transpose(1, 0, 2).reshape(N, N * N)
    return M



# revision 3
# speedup vs baseline: 4.1780x; 4.1780x over previous
"""Trainium2 Bass kernel for nn_Affinity (gnn_message_passing).

M[(a,b),(c,d)] = sum_{j,i} H2[a,j]H2[c,j] H1[b,i]H1[d,i] W[j,i] + diag(Mp),
W[j,i] = X[j] . lamda . Y[i]  (the reference's quirky Me reindexing).

Key structure: block B_{a,c}[b,d] = sum_i H1[b,i]H1[d,i] w_{ac}[i] is itself
sparse -- off-diagonal entries are (multi-edge-merged) values of w_{ac}, and
its diagonal is H1 @ w_{ac}. Folding the 0/1 selection tables (all host-built,
integer-valued) through the algebra, the whole per-core computation reduces to
~18 small matmuls producing two dense value tables:

  WU[u, s] = merged off-diag value of unique graph-1 pair u in slot s's block
  R[b, s]  = diagonal of slot s's block (+ Mp for diagonal slots)

with slots s = the ~360 unique (a,c) block pairs (72 diagonal + unique graph-2
edges), sharded 45-per-core over 8 cores. The host assembles the final
[5184, 5184] matrix by pure index scatter (no host float arithmetic).
"""
import sys
sys.path.insert(0, '/opt/trn_rl_repo')
import numpy as np

N = 72
E = 288
D = 64
NC = 8
S = 48          # slots per core (9 diag + <=36 edge pairs, padded)
UPAD = 288      # unique graph-1 pairs, padded

# PACKA layout [72, 248] bf16
PA = {"l1": (0, 64, 64), "l2": (64, 64, 64), "f1t": (128, 64, 72),
      "csd": (200, 72, S)}
PA_W = 248
# PACKB layout [72, 976] bf16
PB = {"f2": (0, 72, 64), "s2u": (64, 72, UPAD), "d2u": (352, 72, UPAD),
      "s2h": (640, 72, 72), "d2h": (712, 72, 72), "u1t": (784, 64, 72),
      "u2t": (856, 64, 72), "oh": (928, 72, S)}
PB_W = 976


def _split_waits(nc, limit=1):
    """This walrus build rejects instructions with >limit sem waits; move the
    excess onto same-engine NoOps inserted immediately before."""
    import concourse.mybir as mybir
    for f in nc.m.functions:
        for bb in f.blocks:
            new_insts = []
            for inst in bb.instructions:
                si = inst.sync_info
                waits = list(si.on_wait) if si and si.on_wait else []
                if len(waits) > limit:
                    extra, keep = waits[:-limit], waits[-limit:]
                    for i in range(0, len(extra), limit):
                        nop = mybir.InstNoOp(
                            name=nc.get_next_instruction_name(),
                            engine=inst.engine, ins=[], outs=[],
                            sync_info=mybir.SyncInfo(
                                on_wait=extra[i:i + limit], on_update=[]),
                        )
                        nc.register_instruction(nop)
                        new_insts.append(nop)
                    si.on_wait = keep
                new_insts.append(inst)
            bb.instructions[:] = new_insts


def _build_nc():
    import concourse.bass as bass
    import concourse.mybir as mybir
    import concourse.tile as tile

    F32 = mybir.dt.float32
    BF16 = mybir.dt.bfloat16

    nc = bass.Bass()
    pka_d = nc.declare_dram_parameter("PACKA", [72, PA_W], BF16, isOutput=False)
    pkb_d = nc.declare_dram_parameter("PACKB", [72, PB_W], BF16, isOutput=False)
    out_d = nc.declare_dram_parameter("OUT", [96, 4 * S], F32, isOutput=True)

    with tile.TileContext(nc) as tc:
        with tc.tile_pool(name="cst", bufs=1) as cst, \
             tc.tile_pool(name="ps", bufs=1, space="PSUM") as ps:

            pka = cst.tile([72, PA_W], BF16)
            nc.sync.dma_start(out=pka[:], in_=pka_d[:])
            pkb = cst.tile([72, PB_W], BF16)
            nc.scalar.dma_start(out=pkb[:], in_=pkb_d[:])

            def fa(nm):
                o, r, w = PA[nm]
                return pka[0:r, o:o + w]

            def fb(nm):
                o, r, w = PB[nm]
                return pkb[0:r, o:o + w]

            l1, l2, f1t, csd = fa("l1"), fa("l2"), fa("f1t"), fa("csd")
            f2, s2u, d2u = fb("f2"), fb("s2u"), fb("d2u")
            s2h, d2h, u1t, u2t, oh = (fb("s2h"), fb("d2h"), fb("u1t"),
                                      fb("u2t"), fb("oh"))

            # relu(lamda halves)
            r1 = cst.tile([D, D], BF16)
            nc.vector.tensor_relu(out=r1[:], in_=l1)
            r2 = cst.tile([D, D], BF16)
            nc.vector.tensor_relu(out=r2[:], in_=l2)

            # FR = [F1@r1 | F1@r2]  [72, 128]
            fr_p = ps.tile([72, 2 * D], F32, tag="fr")
            nc.tensor.matmul(out=fr_p[:, 0:D], lhsT=f1t, rhs=r1[:],
                             start=True, stop=True)
            nc.tensor.matmul(out=fr_p[:, D:2 * D], lhsT=f1t, rhs=r2[:],
                             start=True, stop=True)
            frb = cst.tile([72, 2 * D], BF16)
            nc.vector.tensor_copy(out=frb[:], in_=fr_p[:])

            # P = [P1 | P2] = [FR1^T@CSD | FR2^T@CSD]  [64, 2S]
            p_p = ps.tile([D, 2 * S], F32, tag="p")
            nc.tensor.matmul(out=p_p[:, 0:S], lhsT=frb[:, 0:D], rhs=csd,
                             start=True, stop=True)
            nc.tensor.matmul(out=p_p[:, S:2 * S], lhsT=frb[:, D:2 * D],
                             rhs=csd, start=True, stop=True)
            pb = cst.tile([D, 2 * S], BF16)
            nc.vector.tensor_copy(out=pb[:], in_=p_p[:])

            # stage-1 feature/table products (independent of the P chain)
            # s1 = [FSU(288) | FSR(72)],  s2 = [FDU(288) | FDR(72)]  [64, 360]
            s1_p = ps.tile([D, UPAD + 72], F32, tag="s1")
            nc.tensor.matmul(out=s1_p[:, 0:UPAD], lhsT=f2, rhs=s2u,
                             start=True, stop=True)
            nc.tensor.matmul(out=s1_p[:, UPAD:UPAD + 72], lhsT=f2, rhs=s2h,
                             start=True, stop=True)
            s2_p = ps.tile([D, UPAD + 72], F32, tag="s2")
            nc.tensor.matmul(out=s2_p[:, 0:UPAD], lhsT=f2, rhs=d2u,
                             start=True, stop=True)
            nc.tensor.matmul(out=s2_p[:, UPAD:UPAD + 72], lhsT=f2, rhs=d2h,
                             start=True, stop=True)
            mp_p = ps.tile([72, 72], F32, tag="mp")
            nc.tensor.matmul(out=mp_p[:], lhsT=u1t, rhs=u2t,
                             start=True, stop=True)
            s1b = cst.tile([D, UPAD + 72], BF16)
            nc.scalar.copy(out=s1b[:], in_=s1_p[:])
            s2b = cst.tile([D, UPAD + 72], BF16)
            nc.vector.tensor_copy(out=s2b[:], in_=s2_p[:])
            mpb = cst.tile([72, 72], BF16)
            nc.scalar.copy(out=mpb[:], in_=mp_p[:])

            # OUT psum [96, 4S]: WU chunks at 0:3S, R at 3S:4S
            o_p = ps.tile([96, 4 * S], F32, tag="o")
            for uc in range(3):
                nc.tensor.matmul(out=o_p[0:96, uc * S:(uc + 1) * S],
                                 lhsT=s1b[:, 96 * uc:96 * (uc + 1)],
                                 rhs=pb[:, 0:S], start=True, stop=False)
                nc.tensor.matmul(out=o_p[0:96, uc * S:(uc + 1) * S],
                                 lhsT=s2b[:, 96 * uc:96 * (uc + 1)],
                                 rhs=pb[:, S:2 * S], start=False, stop=True)
            nc.tensor.matmul(out=o_p[0:72, 3 * S:4 * S],
                             lhsT=s1b[:, UPAD:UPAD + 72], rhs=pb[:, 0:S],
                             start=True, stop=False)
            nc.tensor.matmul(out=o_p[0:72, 3 * S:4 * S],
                             lhsT=s2b[:, UPAD:UPAD + 72], rhs=pb[:, S:2 * S],
                             start=False, stop=False)
            nc.tensor.matmul(out=o_p[0:72, 3 * S:4 * S],
                             lhsT=mpb[:], rhs=oh, start=False, stop=True)

            sbo = cst.tile([96, 4 * S], F32)
            nc.scalar.copy(out=sbo[:], in_=o_p[:])
            nc.sync.dma_start(out=out_d[:], in_=sbo[:])

    _split_waits(nc)
    return nc


def _prepare(inputs):
    import ml_dtypes
    ins = {k: np.asarray(v) for k, v in inputs.items()}
    F1 = ins["F1"].astype(np.float32)
    F2 = ins["F2"].astype(np.float32)
    U1 = ins["U1"].astype(np.float32)
    U2 = ins["U2"].astype(np.float32)
    l1 = ins["lamda1"].astype(np.float32)
    l2 = ins["lamda2"].astype(np.float32)
    src1 = ins["src1"].astype(np.int64)
    dst1 = ins["dst1"].astype(np.int64)
    src2 = ins["src2"].astype(np.int64)
    dst2 = ins["dst2"].astype(np.int64)

    cols = np.arange(E)
    H1 = np.zeros((N, E), np.float32)
    H1[src1, cols] = 1.0
    H1[dst1, cols] = 1.0
    H2 = np.zeros((N, E), np.float32)
    H2[src2, cols] = 1.0
    H2[dst2, cols] = 1.0
    S1 = np.zeros((N, E), np.float32); S1[src1, cols] = 1.0
    D1m = np.zeros((N, E), np.float32); D1m[dst1, cols] = 1.0
    S2 = np.zeros((N, E), np.float32); S2[src2, cols] = 1.0
    D2m = np.zeros((N, E), np.float32); D2m[dst2, cols] = 1.0

    # unique graph-1 pairs (p<q), multi-edges merged; self-loops excluded
    pair_map = {}
    for i in range(E):
        p, q = int(src1[i]), int(dst1[i])
        if p == q:
            continue
        pair_map.setdefault((min(p, q), max(p, q)), []).append(i)
    plist1 = sorted(pair_map)
    G1 = np.zeros((E, UPAD), np.float32)
    for u, key in enumerate(plist1):
        for i in pair_map[key]:
            G1[i, u] = 1.0
    S2U = S2 @ G1
    D2U = D2m @ G1
    S2H = S2 @ H1.T
    D2H = D2m @ H1.T

    # slots: 9 diag per core + unique graph-2 pairs round-robin
    pairs2 = set()
    for j in range(E):
        a, c = int(src2[j]), int(dst2[j])
        if a != c:
            pairs2.add((min(a, c), max(a, c)))
    plist2 = sorted(pairs2)
    core_slots = [[(a, a) for a in range(9 * c, 9 * c + 9)] for c in range(NC)]
    for k, pr in enumerate(plist2):
        core_slots[k % NC].append(pr)
    assert all(len(s) <= S for s in core_slots)

    bf = ml_dtypes.bfloat16
    PACKB = np.zeros((72, PB_W), bf)

    def putb(nm, arr):
        o, r, w = PB[nm]
        PACKB[0:arr.shape[0], o:o + arr.shape[1]] = arr.astype(bf)
    putb("f2", F2)
    putb("s2u", S2U)
    putb("d2u", D2U)
    putb("s2h", S2H)
    putb("d2h", D2H)
    putb("u1t", np.ascontiguousarray(U1.T))
    putb("u2t", np.ascontiguousarray(U2.T))

    base_a = np.zeros((72, PA_W), bf)

    def puta(dst, nm, arr):
        o, r, w = PA[nm]
        dst[0:arr.shape[0], o:o + arr.shape[1]] = arr.astype(bf)
    puta(base_a, "l1", l1)
    puta(base_a, "l2", l2)
    puta(base_a, "f1t", np.ascontiguousarray(F1.T))

    SD1 = S1 + D1m
    in_maps = []
    for c in range(NC):
        slots = core_slots[c]
        SEL = np.zeros((E, S), np.float32)
        OH = np.zeros((N, S), np.float32)
        for s, (a, cc) in enumerate(slots):
            SEL[:, s] = H2[a] * H2[cc]
            if a == cc:
                OH[a, s] = 1.0
        pa = base_a.copy()
        puta(pa, "csd", SD1 @ SEL)
        pb = PACKB.copy()
        putb2 = pb
        o, r, w = PB["oh"]
        putb2[0:N, o:o + S] = OH.astype(bf)
        in_maps.append({"PACKA": pa, "PACKB": pb})
    aux = {"plist1": plist1, "core_slots": core_slots}
    return in_maps, aux


_CACHE = {}


def _get_nc():
    nc = _CACHE.get("nc")
    if nc is None:
        nc = _build_nc()
        _CACHE["nc"] = nc
    return nc


def kernel(**inputs):
    from concourse.bass_utils import run_bass_kernel_spmd

    in_maps, aux = _prepare(inputs)
    nc = _get_nc()
    res = run_bass_kernel_spmd(nc, in_maps, list(range(NC)))

    plist1 = aux["plist1"]
    nu = len(plist1)
    pu = np.array([p for p, q in plist1], np.int64)
    qu = np.array([q for p, q in plist1], np.int64)
    t = np.arange(N)
    M = np.zeros((N * N, N * N), np.float32)
    for c in range(NC):
        out = res.results[c]["OUT"]
        WU = np.concatenate([out[0:96, 0:S], out[0:96, S:2 * S],
                             out[0:96, 2 * S:3 * S]], axis=0)  # [288, S]
        R = out[0:N, 3 * S:4 * S]                               # [72, S]
        for s, (a, cc) in enumerate(aux["core_slots"][c]):
            v = WU[0:nu, s]
            M[a * N + pu, cc * N + qu] = v
            M[a * N + qu, cc * N + pu] = v
            M[a * N + t, cc * N + t] = R[:, s]
            if a != cc:
                M[cc * N + pu, a * N + qu] = v
                M[cc * N + qu, a * N + pu] = v
                M[cc * N + t, a * N + t] = R[:, s]
    return M
